# revision 1
# baseline (speedup 1.0000x reference)
"""CanonGLU feedforward layer on 8 TRN2 NeuronCores.

Math (per reference):
    gate = x @ w_gate.T ; up = x @ w_up.T            # [B,T,F]
    gate += causal_dconv(gate, conv_w[:F]) ; up += causal_dconv(up, conv_w[F:])
    out  = (up * silu(gate)) @ w_down.T              # [B,T,D]

Sharding: tensor-parallel over d_ff. Core c owns f-slice [c*1024,(c+1)*1024) of
w_gate/w_up/conv_w (column parallel) and w_down (row parallel); x replicated.
Each core computes a full-shape partial output; the host sums the 8 partials
(the "all-reduce").

Per-core layout: everything keeps d_ff (f) or d_model (d) on SBUF partitions
and tokens (t) on the free axis, so the depthwise conv is per-partition
scalar ops with shifted free-dim slices.  Matmul operands are fp16 (PE runs
fp16 at 1 cyc/row vs 4 for fp32; host-measured absmax/scale error 5e-4),
accumulation stays fp32 in PSUM.
"""

import numpy as np

import concourse.bass as bass
import concourse.mybir as mybir
import concourse.tile as tile
from concourse import bacc
from concourse.bass_utils import run_bass_kernel_spmd

F16 = mybir.dt.float16
F32 = mybir.dt.float32
AF = mybir.ActivationFunctionType
ALU = mybir.AluOpType

B, T, D, F = 2, 2048, 2048, 8192
NCORES = 8
FC_PER_CORE = F // NCORES          # 1024 f per core
TT = B * T                         # 4096 tokens total
NT = 512                           # token tile (one PSUM bank of fp32)
N_TILES = TT // NT                 # 8
TILES_PER_BATCH = T // NT          # 4 (conv halo resets at 0 and 4)
DC = D // 128                      # 16 d-chunks
FC = FC_PER_CORE // 128            # 8 f-chunks per core
GROW = NT + 4                      # conv buffer row: 3 halo + 512 data + 1 pad


def build_nc():
    nc = bacc.Bacc(None, target_bir_lowering=False, debug=False)

    xT = nc.dram_tensor("xT", [D, TT], F16, kind="ExternalInput")
    wgT = nc.dram_tensor("wgT", [D, FC_PER_CORE], F16, kind="ExternalInput")
    wuT = nc.dram_tensor("wuT", [D, FC_PER_CORE], F16, kind="ExternalInput")
    wdT = nc.dram_tensor("wdT", [FC_PER_CORE, D], F16, kind="ExternalInput")
    cw = nc.dram_tensor("cw", [128, FC, 2, 4], F32, kind="ExternalInput")
    outT = nc.dram_tensor("outT", [D, TT], F32, kind="ExternalOutput")

    xTr = xT.rearrange("(dc p) t -> p dc t", p=128)
    wgTr = wgT.rearrange("(dc p) f -> p dc f", p=128)
    wuTr = wuT.rearrange("(dc p) f -> p dc f", p=128)
    wdTr = wdT.rearrange("(fc p) d -> p fc d", p=128)

    with tile.TileContext(nc) as tc:
        with (
            tc.tile_pool(name="consts", bufs=1) as consts,
            tc.tile_pool(name="xp", bufs=2) as xpool,
            tc.tile_pool(name="gb", bufs=2) as gbpool,
            tc.tile_pool(name="ub", bufs=2) as ubpool,
            tc.tile_pool(name="hp", bufs=2 * FC) as hpool,
            tc.tile_pool(name="tp", bufs=6) as tpool,
            tc.tile_pool(name="op", bufs=3) as opool,
            tc.tile_pool(name="psg", bufs=2, space="PSUM") as ps_g,
            tc.tile_pool(name="psu", bufs=2, space="PSUM") as ps_u,
            tc.tile_pool(name="pso", bufs=4, space="PSUM") as ps_o,
        ):
            wg_sb = consts.tile([128, DC, FC_PER_CORE], F16)
            wu_sb = consts.tile([128, DC, FC_PER_CORE], F16)
            wd_sb = consts.tile([128, FC, D], F16)
            cw_sb = consts.tile([128, FC, 2, 4], F32)
            zero_b = consts.tile([128, 1], F32)
            nc.vector.memset(zero_b[:], 0.0)

            x_tiles = {}

            def load_x(tt, split=1):
                x_t = xpool.tile([128, DC, NT], F16)
                step = DC // split
                for i in range(split):
                    sl = slice(i * step, (i + 1) * step)
                    nc.sync.dma_start(
                        out=x_t[:, sl, :],
                        in_=xTr[:, sl, bass.ts(tt, NT)])
                x_tiles[tt] = x_t

            # DMA order matters at startup: x(0) first so the PE can start,
            # then gate/up weights per-chunk (matmuls start as chunks land),
            # conv weights, and w_down last (first needed ~54us in).
            # PE warmup: dummy matmuls on zeroed SBUF fill the startup DMA
            # wait so the HAM clock gate reaches (and keeps) 2.4 GHz before
            # the first real matmul.
            warm_sb = consts.tile([128, NT], F16)
            nc.gpsimd.memset(warm_sb[:], 0.0)
            warm_ps = ps_g.tile([128, NT], F32, tag="ps")
            for _ in range(24):
                nc.tensor.matmul(
                    warm_ps[:], warm_sb[:, 0:128], warm_sb[:],
                    start=True, stop=True)

            load_x(0, split=8)
            # gate consumes w_gate first; all of w_gate before any w_up so the
            # first gate psum group isn't gated on the tail of the interleave.
            for dc in range(DC):
                nc.sync.dma_start(out=wg_sb[:, dc, :], in_=wgTr[:, dc, :])
            for dc in range(DC):
                nc.sync.dma_start(out=wu_sb[:, dc, :], in_=wuTr[:, dc, :])
            nc.sync.dma_start(out=cw_sb[:], in_=cw[:])
            for fc in range(FC):
                nc.sync.dma_start(out=wd_sb[:, fc, :], in_=wdTr[:, fc, :])

            g_prev = u_prev = None
            h_tiles = {}

            def gateup_phase(tt):
                nonlocal g_prev, u_prev
                x_t = x_tiles[tt]
                g_cur = gbpool.tile([128, FC, GROW], F16)
                u_cur = ubpool.tile([128, FC, GROW], F16)
                hs = []
                for br, (w_sb, buf, prev, psp) in enumerate((
                    (wg_sb, g_cur, g_prev, ps_g),
                    (wu_sb, u_cur, u_prev, ps_u),
                )):
                    # conv halo: last 3 tokens of the previous tile (zeros at
                    # the start of each batch — causal left pad).
                    if tt % TILES_PER_BATCH == 0:
                        nc.vector.memset(buf[:, :, 0:3], 0.0)
                    else:
                        nc.vector.tensor_copy(
                            out=buf[:, :, 0:3], in_=prev[:, :, NT:NT + 3])
                    for fc in range(FC):
                        ps = psp.tile([128, NT], F32)
                        for dc in range(DC):
                            nc.tensor.matmul(
                                ps[:],
                                w_sb[:, dc, bass.ts(fc, 128)],
                                x_t[:, dc, :],
                                start=(dc == 0),
                                stop=(dc == DC - 1),
                            )
                        nc.scalar.copy(out=buf[:, fc, 3:3 + NT], in_=ps[:])
                for fc in range(FC):
                    # causal depthwise conv + residual (folded into tap 3),
                    # then h = up_conv * silu(gate_conv).
                    conv_out = []
                    for br, buf in ((0, g_cur), (1, u_cur)):
                        t1 = tpool.tile([128, NT], F16)
                        # All taps on VectorE so ScalarE's stream stays
                        # homogeneous (Copy evacs, then Sigmoids) — avoids
                        # per-op activation-table reloads on HW.
                        nc.vector.tensor_scalar(
                            t1[:], buf[:, fc, 1:1 + NT],
                            cw_sb[:, fc, br, 1:2], None, ALU.mult)
                        for k in (0, 2, 3):
                            nc.vector.scalar_tensor_tensor(
                                out=t1[:], in0=buf[:, fc, k:k + NT],
                                scalar=cw_sb[:, fc, br, k:k + 1], in1=t1[:],
                                op0=ALU.mult, op1=ALU.add)
                        conv_out.append(t1)
                    gc, uc = conv_out
                    sg = tpool.tile([128, NT], F16)
                    nc.scalar.activation(
                        out=sg[:], in_=gc[:], func=AF.Sigmoid,
                        bias=zero_b[:, 0:1])
                    nc.vector.tensor_mul(sg[:], sg[:], gc[:])
                    h_t = hpool.tile([128, NT], F16)
                    nc.vector.tensor_mul(h_t[:], uc[:], sg[:])
                    hs.append(h_t)
                g_prev, u_prev = g_cur, u_cur
                h_tiles[tt] = hs

            def down_phase(tt):
                hs = h_tiles.pop(tt)
                for dc in range(DC):
                    pso = ps_o.tile([128, NT], F32)
                    for fc in range(FC):
                        nc.tensor.matmul(
                            pso[:],
                            wd_sb[:, fc, bass.ts(dc, 128)],
                            hs[fc][:],
                            start=(fc == 0),
                            stop=(fc == FC - 1),
                        )
                    o_sb = opool.tile([128, NT], F32)
                    nc.scalar.copy(out=o_sb[:], in_=pso[:])
                    # alternate the two HWDGE queues so the final tile's
                    # output drain is not serialized behind one queue
                    eng = nc.sync if dc % 2 == 0 else nc.scalar
                    eng.dma_start(
                        out=outT[bass.ts(dc, 128), bass.ts(tt, NT)],
                        in_=o_sb[:])

            # Software pipeline: gate/up(tt) is emitted before down(tt-1) so
            # the PE never waits on the conv/act chain of the current tile.
            for tt in range(N_TILES + 1):
                if tt + 1 <= N_TILES - 1:
                    load_x(tt + 1)
                if tt < N_TILES:
                    gateup_phase(tt)
                if tt >= 1:
                    down_phase(tt - 1)

    nc.compile()
    return nc


_NC_CACHE = None


def _get_nc():
    global _NC_CACHE
    if _NC_CACHE is None:
        _NC_CACHE = build_nc()
    return _NC_CACHE


def _prep_inputs(x, w_gate, w_up, w_down, conv_w):
    xT = np.ascontiguousarray(
        x.reshape(TT, D).T).astype(np.float16)         # [D, TT]
    # conv weights: [2F, 4] -> per-core [128, FC, 2, 4], residual folded in
    cwf = conv_w.reshape(2, NCORES, FC, 128, 4).astype(np.float32)
    in_maps = []
    for c in range(NCORES):
        fs = slice(c * FC_PER_CORE, (c + 1) * FC_PER_CORE)
        wgT = np.ascontiguousarray(w_gate[fs].T).astype(np.float16)
        wuT = np.ascontiguousarray(w_up[fs].T).astype(np.float16)
        wdT = np.ascontiguousarray(w_down[:, fs].T).astype(np.float16)
        cwc = np.ascontiguousarray(
            cwf[:, c].transpose(2, 1, 0, 3))           # [128, FC, 2, 4]
        cwc[:, :, :, 3] += 1.0
        in_maps.append({"xT": xT, "wgT": wgT, "wuT": wuT, "wdT": wdT,
                        "cw": cwc})
    return in_maps


def run_spmd(in_maps, **kwargs):
    nc = _get_nc()
    return run_bass_kernel_spmd(
        nc, in_maps, core_ids=list(range(NCORES)), **kwargs)


def kernel(x, w_gate, w_up, w_down, conv_w):
    in_maps = _prep_inputs(
        np.asarray(x, dtype=np.float32), np.asarray(w_gate, dtype=np.float32),
        np.asarray(w_up, dtype=np.float32),
        np.asarray(w_down, dtype=np.float32),
        np.asarray(conv_w, dtype=np.float32))
    res = run_spmd(in_maps)
    acc = np.zeros((D, TT), np.float32)
    for r in res.results:
        acc += r["outT"]
    return np.ascontiguousarray(acc.T).reshape(B, T, D)



# revision 4
# speedup vs baseline: 1.2938x; 1.2938x over previous
"""CanonGLU feedforward layer on 8 TRN2 NeuronCores.

Math (per reference):
    gate = x @ w_gate.T ; up = x @ w_up.T            # [B,T,F]
    gate += causal_dconv(gate, conv_w[:F]) ; up += causal_dconv(up, conv_w[F:])
    out  = (up * silu(gate)) @ w_down.T              # [B,T,D]

Sharding: tensor-parallel over d_ff. Core c owns f-slice [c*1024,(c+1)*1024) of
w_gate/w_up/conv_w (column parallel) and w_down (row parallel); x replicated.
Each core computes a full-shape partial output; the host sums the 8 partials
(the "all-reduce").

Matmuls run as fp8e4 (e4m3) DoubleRow pairs: each instruction contracts two
128-row planes at 0.5 cyc/output-column -- 4x the fp16 row rate.  Accuracy is
recovered with a hi/lo split: every operand X is stored as X_h = fp8(X*s) plus
X_l = fp8(X*s - X_h) at the SAME scale, and each product uses three terms
 A_h@B_h + A_l@B_h + A_h@B_l  (the lo*lo term is ~0.06% and dropped), so each
projection costs 0.75x its fp16 cycle count while the end-to-end error stays
~2e-3 (measured on host emulation).  x and the weights are split on the host;
h = up_c * silu(gate_c) is split on-device (1 scaled copy + 1 subtract per
f-chunk).  PSUM accumulates fp32; conv/silu stay fp16 at true scale (the
fp8 scale product is divided out in the PSUM-evacuation copy).
"""

import numpy as np
import ml_dtypes

import concourse.bass as bass
import concourse.mybir as mybir
import concourse.tile as tile
from concourse import bacc
from concourse.bass_utils import run_bass_kernel_spmd

F8 = mybir.dt.float8e4
F16 = mybir.dt.float16
F32 = mybir.dt.float32
AF = mybir.ActivationFunctionType
ALU = mybir.AluOpType
DR = mybir.MatmulPerfMode.DoubleRow

B, T, D, F = 2, 2048, 2048, 8192
NCORES = 8
FC_PER_CORE = F // NCORES          # 1024 f per core
TT = B * T                         # 4096 tokens total
NT = 512                           # token tile (one PSUM bank of fp32)
NH = NT // 2                       # DoubleRow moving free limit: 2*NH <= 512
N_TILES = TT // NT                 # 8
TILES_PER_BATCH = T // NT          # 4 (conv halo resets at 0 and 4)
DC2 = D // 256                     # 8 d plane-pairs
FC = FC_PER_CORE // 128            # 8 f-chunks per core
FC2 = FC // 2                      # 4 f plane-pairs per core
GROW = NT + 4                      # conv buffer row: 3 halo + 512 data + 1 pad

SX = 16.0                          # x fp8 scale
SW = 512.0                         # weight fp8 scale
SH = 4.0                           # h fp8 scale
E4 = ml_dtypes.float8_e4m3


def build_nc():
    nc = bacc.Bacc(None, target_bir_lowering=False, debug=False)

    # hi/lo fp8 pairs, prepared on the host at a shared power-of-2 scale
    xh = nc.dram_tensor("xh", [D, TT], F8, kind="ExternalInput")
    xl = nc.dram_tensor("xl", [D, TT], F8, kind="ExternalInput")
    wgh = nc.dram_tensor("wgh", [D, FC_PER_CORE], F8, kind="ExternalInput")
    wgl = nc.dram_tensor("wgl", [D, FC_PER_CORE], F8, kind="ExternalInput")
    wuh = nc.dram_tensor("wuh", [D, FC_PER_CORE], F8, kind="ExternalInput")
    wul = nc.dram_tensor("wul", [D, FC_PER_CORE], F8, kind="ExternalInput")
    wdh = nc.dram_tensor("wdh", [FC_PER_CORE, D], F8, kind="ExternalInput")
    wdl = nc.dram_tensor("wdl", [FC_PER_CORE, D], F8, kind="ExternalInput")
    cw = nc.dram_tensor("cw", [128, FC, 2, 4], F32, kind="ExternalInput")
    outT = nc.dram_tensor("outT", [D, TT], F16, kind="ExternalOutput")

    # DoubleRow plane pairing: contraction index = 256*c + 128*i + partition
    xhr = xh.rearrange("(c i p) t -> p c i t", p=128, i=2)
    xlr = xl.rearrange("(c i p) t -> p c i t", p=128, i=2)
    wr = {
        "gh": wgh.rearrange("(c i p) f -> p c i f", p=128, i=2),
        "gl": wgl.rearrange("(c i p) f -> p c i f", p=128, i=2),
        "uh": wuh.rearrange("(c i p) f -> p c i f", p=128, i=2),
        "ul": wul.rearrange("(c i p) f -> p c i f", p=128, i=2),
    }
    wdhr = wdh.rearrange("(c i p) d -> p c i d", p=128, i=2)
    wdlr = wdl.rearrange("(c i p) d -> p c i d", p=128, i=2)

    with tile.TileContext(nc) as tc:
        with (
            tc.tile_pool(name="consts", bufs=1) as consts,
            tc.tile_pool(name="xp", bufs=2) as xpool,
            tc.tile_pool(name="gb", bufs=2) as gbpool,
            tc.tile_pool(name="ub", bufs=2) as ubpool,
            tc.tile_pool(name="hp", bufs=2 * FC2) as hpool,
            tc.tile_pool(name="tp", bufs=4) as tpool,
            tc.tile_pool(name="op", bufs=3) as opool,
            tc.tile_pool(name="psg", bufs=2, space="PSUM") as ps_g,
            tc.tile_pool(name="psu", bufs=2, space="PSUM") as ps_u,
            tc.tile_pool(name="pso", bufs=4, space="PSUM") as ps_o,
        ):
            wgh_sb = consts.tile([128, DC2, 2, FC_PER_CORE], F8)
            wgl_sb = consts.tile([128, DC2, 2, FC_PER_CORE], F8)
            wuh_sb = consts.tile([128, DC2, 2, FC_PER_CORE], F8)
            wul_sb = consts.tile([128, DC2, 2, FC_PER_CORE], F8)
            w_sb = {"gh": wgh_sb, "gl": wgl_sb, "uh": wuh_sb, "ul": wul_sb}
            wdh_sb = consts.tile([128, FC2, 2, D], F8)
            wdl_sb = consts.tile([128, FC2, 2, D], F8)
            cw_sb = consts.tile([128, FC, 2, 4], F32)
            zero_b = consts.tile([128, 1], F32)
            nc.vector.memset(zero_b[:], 0.0)

            x_tiles = {}

            def load_x(tt, split=1):
                xh_t = xpool.tile([128, DC2, 2, NT], F8)
                xl_t = xpool.tile([128, DC2, 2, NT], F8)
                step = DC2 // split
                for i in range(split):
                    sl = slice(i * step, (i + 1) * step)
                    nc.sync.dma_start(
                        out=xh_t[:, sl, :, :],
                        in_=xhr[:, sl, :, bass.ts(tt, NT)])
                    nc.sync.dma_start(
                        out=xl_t[:, sl, :, :],
                        in_=xlr[:, sl, :, bass.ts(tt, NT)])
                x_tiles[tt] = (xh_t, xl_t)

            # DMA order at startup: x(0) first, then gate hi/lo weights
            # per-chunk (matmuls start as chunks land), up weights, conv
            # weights, w_down last (first needed ~50us in).  PE warmup:
            # dummy matmuls on zeroed SBUF fill the startup DMA wait so the
            # clock gate reaches (and keeps) 2.4 GHz before the first real
            # matmul.
            warm_sb = consts.tile([128, NT], F16)
            nc.gpsimd.memset(warm_sb[:], 0.0)
            warm_ps = ps_g.tile([128, NT], F32, tag="ps")
            for _ in range(24):
                nc.tensor.matmul(
                    warm_ps[:], warm_sb[:, 0:128], warm_sb[:],
                    start=True, stop=True)

            load_x(0, split=8)
            for c in range(DC2):
                nc.sync.dma_start(out=w_sb["gh"][:, c, :, :],
                                  in_=wr["gh"][:, c, :, :])
                nc.sync.dma_start(out=w_sb["gl"][:, c, :, :],
                                  in_=wr["gl"][:, c, :, :])
            for c in range(DC2):
                nc.sync.dma_start(out=w_sb["uh"][:, c, :, :],
                                  in_=wr["uh"][:, c, :, :])
                nc.sync.dma_start(out=w_sb["ul"][:, c, :, :],
                                  in_=wr["ul"][:, c, :, :])
            nc.sync.dma_start(out=cw_sb[:], in_=cw[:])
            for c in range(FC2):
                nc.sync.dma_start(out=wdh_sb[:, c, :, :], in_=wdhr[:, c, :, :])
                nc.sync.dma_start(out=wdl_sb[:, c, :, :], in_=wdlr[:, c, :, :])

            g_prev = u_prev = None
            h_tiles = {}

            def gu_matmul(ps, w_hi, w_lo, x_t, fc):
                """One f-chunk of gate/up: 3-term hi/lo fp8 DoubleRow."""
                xh_t, xl_t = x_t
                fs = bass.ts(fc, 128)
                for half in range(2):
                    ts = bass.ts(half, NH)
                    n = 0
                    for rhs_t, lhsT in ((xh_t, w_hi), (xh_t, w_lo),
                                        (xl_t, w_hi)):
                        for c in range(DC2):
                            nc.tensor.matmul(
                                ps[:, ts],
                                lhsT[:, c, :, fs],
                                rhs_t[:, c, :, ts],
                                start=(n == 0),
                                stop=(n == 3 * DC2 - 1),
                                perf_mode=DR,
                            )
                            n += 1

            def gateup_phase(tt):
                nonlocal g_prev, u_prev
                x_t = x_tiles[tt]
                g_cur = gbpool.tile([128, FC, GROW], F16)
                u_cur = ubpool.tile([128, FC, GROW], F16)
                hs = []
                for br, (hi, lo, buf, prev, psp) in enumerate((
                    ("gh", "gl", g_cur, g_prev, ps_g),
                    ("uh", "ul", u_cur, u_prev, ps_u),
                )):
                    # conv halo: last 3 tokens of the previous tile (zeros at
                    # the start of each batch — causal left pad).
                    if tt % TILES_PER_BATCH == 0:
                        nc.vector.memset(buf[:, :, 0:3], 0.0)
                    else:
                        nc.vector.tensor_copy(
                            out=buf[:, :, 0:3], in_=prev[:, :, NT:NT + 3])
                    for fc in range(FC):
                        ps = psp.tile([128, NT], F32)
                        gu_matmul(ps, w_sb[hi], w_sb[lo], x_t, fc)
                        # evacuate and divide out the fp8 scale product
                        nc.scalar.mul(buf[:, fc, 3:3 + NT], ps[:],
                                      1.0 / (SX * SW))
                for c2 in range(FC2):
                    hh_t = hpool.tile([128, 2, NT], F8)
                    hl_t = hpool.tile([128, 2, NT], F8)
                    for i in range(2):
                        fc = 2 * c2 + i
                        # causal depthwise conv + residual (folded into tap
                        # 3), then h = up_conv * silu(gate_conv).
                        conv_out = []
                        for br, buf in ((0, g_cur), (1, u_cur)):
                            t1 = tpool.tile([128, NT], F16)
                            # All taps on VectorE so ScalarE's stream stays
                            # homogeneous (Copy evacs, then Sigmoids).
                            nc.vector.tensor_scalar(
                                t1[:], buf[:, fc, 1:1 + NT],
                                cw_sb[:, fc, br, 1:2], None, ALU.mult)
                            for k in (0, 2, 3):
                                nc.vector.scalar_tensor_tensor(
                                    out=t1[:], in0=buf[:, fc, k:k + NT],
                                    scalar=cw_sb[:, fc, br, k:k + 1], in1=t1[:],
                                    op0=ALU.mult, op1=ALU.add)
                            conv_out.append(t1)
                        gc, uc = conv_out
                        sg = tpool.tile([128, NT], F16)
                        nc.scalar.activation(
                            out=sg[:], in_=gc[:], func=AF.Sigmoid,
                            bias=zero_b[:, 0:1])
                        nc.vector.tensor_mul(sg[:], sg[:], gc[:])
                        h_t = tpool.tile([128, NT], F16)
                        nc.vector.tensor_mul(h_t[:], uc[:], sg[:])
                        # hi/lo fp8 split of h at scale SH (same scale for
                        # both so all 3 down-proj terms share one PSUM group)
                        nc.scalar.mul(hh_t[:, i, :], h_t[:], SH)
                        nc.vector.scalar_tensor_tensor(
                            out=hl_t[:, i, :], in0=h_t[:], scalar=SH,
                            in1=hh_t[:, i, :],
                            op0=ALU.mult, op1=ALU.subtract)
                    hs.append((hh_t, hl_t))
                g_prev, u_prev = g_cur, u_cur
                h_tiles[tt] = hs

            def down_phase(tt):
                hs = h_tiles.pop(tt)
                for dc in range(D // 128):
                    ds = bass.ts(dc, 128)
                    pso = ps_o.tile([128, NT], F32)
                    for half in range(2):
                        ts = bass.ts(half, NH)
                        n = 0
                        for sel_h, wd_t in ((0, wdh_sb), (0, wdl_sb),
                                            (1, wdh_sb)):
                            for c in range(FC2):
                                nc.tensor.matmul(
                                    pso[:, ts],
                                    wd_t[:, c, :, ds],
                                    hs[c][sel_h][:, :, ts],
                                    start=(n == 0),
                                    stop=(n == 3 * FC2 - 1),
                                    perf_mode=DR,
                                )
                                n += 1
                    o_sb = opool.tile([128, NT], F16)
                    nc.scalar.mul(o_sb[:], pso[:], 1.0 / (SH * SW))
                    # alternate the two HWDGE queues so the final tile's
                    # output drain is not serialized behind one queue
                    eng = nc.sync if dc % 2 == 0 else nc.scalar
                    eng.dma_start(
                        out=outT[ds, bass.ts(tt, NT)],
                        in_=o_sb[:])

            # Software pipeline: gate/up(tt) is emitted before down(tt-1) so
            # the PE never waits on the conv/act chain of the current tile.
            for tt in range(N_TILES + 1):
                if tt + 1 <= N_TILES - 1:
                    load_x(tt + 1)
                if tt < N_TILES:
                    gateup_phase(tt)
                if tt >= 1:
                    down_phase(tt - 1)

    nc.compile()
    return nc


_NC_CACHE = None


def _get_nc():
    global _NC_CACHE
    if _NC_CACHE is None:
        _NC_CACHE = build_nc()
    return _NC_CACHE


def _split8(a, scale):
    """hi/lo fp8e4 pair of a*scale (shared scale; lo = quantized residual)."""
    sa = a * scale
    hi = sa.astype(E4)
    lo = (sa - hi.astype(np.float32)).astype(E4)
    return hi, lo


def _prep_inputs(x, w_gate, w_up, w_down, conv_w):
    xT = np.ascontiguousarray(x.reshape(TT, D).T)      # [D, TT] fp32
    xh_a, xl_a = _split8(xT, SX)
    # conv weights: [2F, 4] -> per-core [128, FC, 2, 4], residual folded in
    cwf = conv_w.reshape(2, NCORES, FC, 128, 4).astype(np.float32)
    in_maps = []
    for c in range(NCORES):
        fs = slice(c * FC_PER_CORE, (c + 1) * FC_PER_CORE)
        wgh_a, wgl_a = _split8(np.ascontiguousarray(w_gate[fs].T), SW)
        wuh_a, wul_a = _split8(np.ascontiguousarray(w_up[fs].T), SW)
        wdh_a, wdl_a = _split8(np.ascontiguousarray(w_down[:, fs].T), SW)
        cwc = np.ascontiguousarray(
            cwf[:, c].transpose(2, 1, 0, 3))           # [128, FC, 2, 4]
        cwc[:, :, :, 3] += 1.0
        in_maps.append({"xh": xh_a, "xl": xl_a,
                        "wgh": wgh_a, "wgl": wgl_a,
                        "wuh": wuh_a, "wul": wul_a,
                        "wdh": wdh_a, "wdl": wdl_a,
                        "cw": cwc})
    return in_maps


def run_spmd(in_maps, **kwargs):
    nc = _get_nc()
    return run_bass_kernel_spmd(
        nc, in_maps, core_ids=list(range(NCORES)), **kwargs)


def kernel(x, w_gate, w_up, w_down, conv_w):
    in_maps = _prep_inputs(
        np.asarray(x, dtype=np.float32), np.asarray(w_gate, dtype=np.float32),
        np.asarray(w_up, dtype=np.float32),
        np.asarray(w_down, dtype=np.float32),
        np.asarray(conv_w, dtype=np.float32))
    res = run_spmd(in_maps)
    acc = np.zeros((D, TT), np.float32)
    for r in res.results:
        acc += r["outT"].astype(np.float32)
    return np.ascontiguousarray(acc.T).reshape(B, T, D)


# revision 13
# speedup vs baseline: 1.3286x; 1.0269x over previous
"""CanonGLU feedforward layer on 8 TRN2 NeuronCores.

Math (per reference):
    gate = x @ w_gate.T ; up = x @ w_up.T            # [B,T,F]
    gate += causal_dconv(gate, conv_w[:F]) ; up += causal_dconv(up, conv_w[F:])
    out  = (up * silu(gate)) @ w_down.T              # [B,T,D]

Sharding: tensor-parallel over d_ff. Core c owns f-slice [c*1024,(c+1)*1024) of
w_gate/w_up/conv_w (column parallel) and w_down (row parallel); x replicated.
Each core computes a full-shape partial output; the host sums the 8 partials
(the "all-reduce").

Matmuls run as fp8e4 (e4m3) DoubleRow pairs: each instruction contracts two
128-row planes at 0.5 cyc/output-column -- 4x the fp16 row rate.  Accuracy is
recovered with a hi/lo split: every operand X is stored as X_h = fp8(X*s) plus
X_l = fp8(X*s - X_h) at the SAME scale, and each product uses three terms
 A_h@B_h + A_l@B_h + A_h@B_l  (the lo*lo term is ~0.06% and dropped), so each
projection costs 0.75x its fp16 cycle count while the end-to-end error stays
~2e-3 (measured on host emulation).  x and the weights are split on the host;
h = up_c * silu(gate_c) is split on-device (1 scaled copy + 1 subtract per
f-chunk).  PSUM accumulates fp32; conv/silu stay fp16 at true scale (the
fp8 scale product is divided out in the PSUM-evacuation copy).
"""

import numpy as np
import ml_dtypes

import concourse.bass as bass
import concourse.mybir as mybir
import concourse.tile as tile
from concourse import bacc
from concourse.bass_utils import run_bass_kernel_spmd

F8 = mybir.dt.float8e4
F16 = mybir.dt.float16
F32 = mybir.dt.float32
AF = mybir.ActivationFunctionType
ALU = mybir.AluOpType
DR = mybir.MatmulPerfMode.DoubleRow

B, T, D, F = 2, 2048, 2048, 8192
NCORES = 8
FC_PER_CORE = F // NCORES          # 1024 f per core
TT = B * T                         # 4096 tokens total
NT = 512                           # token tile (one PSUM bank of fp32)
NH = NT // 2                       # DoubleRow moving free limit: 2*NH <= 512
N_TILES = TT // NT                 # 8
TILES_PER_BATCH = T // NT          # 4 (conv halo resets at 0 and 4)
DC2 = D // 256                     # 8 d plane-pairs
FC = FC_PER_CORE // 128            # 8 f-chunks per core
FC2 = FC // 2                      # 4 f plane-pairs per core
GROW = NT + 4                      # conv buffer row: 3 halo + 512 data + 1 pad

SX = 16.0                          # x fp8 scale
SW = 512.0                         # weight fp8 scale
SH = 4.0                           # h fp8 scale
E4 = ml_dtypes.float8_e4m3


def build_nc():
    nc = bacc.Bacc(None, target_bir_lowering=False, debug=False)

    # hi/lo fp8 pairs, prepared on the host at a shared power-of-2 scale
    xh = nc.dram_tensor("xh", [D, TT], F8, kind="ExternalInput")
    xl = nc.dram_tensor("xl", [D, TT], F8, kind="ExternalInput")
    wgh = nc.dram_tensor("wgh", [D, FC_PER_CORE], F8, kind="ExternalInput")
    wgl = nc.dram_tensor("wgl", [D, FC_PER_CORE], F8, kind="ExternalInput")
    wuh = nc.dram_tensor("wuh", [D, FC_PER_CORE], F8, kind="ExternalInput")
    wul = nc.dram_tensor("wul", [D, FC_PER_CORE], F8, kind="ExternalInput")
    wdh = nc.dram_tensor("wdh", [FC_PER_CORE, D], F8, kind="ExternalInput")
    wdl = nc.dram_tensor("wdl", [FC_PER_CORE, D], F8, kind="ExternalInput")
    cw = nc.dram_tensor("cw", [128, FC, 2, 4], F32, kind="ExternalInput")
    outT = nc.dram_tensor("outT", [D, TT], F16, kind="ExternalOutput")

    # DoubleRow plane pairing: contraction index = 256*c + 128*i + partition
    xhr = xh.rearrange("(c i p) t -> p c i t", p=128, i=2)
    xlr = xl.rearrange("(c i p) t -> p c i t", p=128, i=2)
    wr = {
        "gh": wgh.rearrange("(c i p) f -> p c i f", p=128, i=2),
        "gl": wgl.rearrange("(c i p) f -> p c i f", p=128, i=2),
        "uh": wuh.rearrange("(c i p) f -> p c i f", p=128, i=2),
        "ul": wul.rearrange("(c i p) f -> p c i f", p=128, i=2),
    }
    wdhr = wdh.rearrange("(c i p) d -> p c i d", p=128, i=2)
    wdlr = wdl.rearrange("(c i p) d -> p c i d", p=128, i=2)

    with tile.TileContext(nc) as tc:
        with (
            tc.tile_pool(name="consts", bufs=1) as consts,
            tc.tile_pool(name="xp", bufs=2) as xpool,
            tc.tile_pool(name="gb", bufs=2) as gbpool,
            tc.tile_pool(name="ub", bufs=2) as ubpool,
            tc.tile_pool(name="hp", bufs=2 * FC2) as hpool,
            tc.tile_pool(name="tp", bufs=4) as tpool,
            tc.tile_pool(name="op", bufs=3) as opool,
            tc.tile_pool(name="psg", bufs=2, space="PSUM") as ps_g,
            tc.tile_pool(name="psu", bufs=2, space="PSUM") as ps_u,
            tc.tile_pool(name="pso", bufs=4, space="PSUM") as ps_o,
        ):
            wgh_sb = consts.tile([128, DC2, 2, FC_PER_CORE], F8)
            wgl_sb = consts.tile([128, DC2, 2, FC_PER_CORE], F8)
            wuh_sb = consts.tile([128, DC2, 2, FC_PER_CORE], F8)
            wul_sb = consts.tile([128, DC2, 2, FC_PER_CORE], F8)
            w_sb = {"gh": wgh_sb, "gl": wgl_sb, "uh": wuh_sb, "ul": wul_sb}
            wdh_sb = consts.tile([128, FC2, 2, D], F8)
            wdl_sb = consts.tile([128, FC2, 2, D], F8)
            cw_sb = consts.tile([128, FC, 2, 4], F32)
            zero_b = consts.tile([128, 1], F32)
            nc.vector.memset(zero_b[:], 0.0)

            x_tiles = {}

            def load_x(tt):
                xh_t = xpool.tile([128, DC2, 2, NT], F8)
                xl_t = xpool.tile([128, DC2, 2, NT], F8)
                nc.sync.dma_start(out=xh_t[:], in_=xhr[:, :, :, bass.ts(tt, NT)])
                nc.sync.dma_start(out=xl_t[:], in_=xlr[:, :, :, bass.ts(tt, NT)])
                x_tiles[tt] = (xh_t, xl_t)

            # DMA order at startup: x(0) and the gate hi/lo weights first
            # (first gate group is gated by these — big transfers, since the
            # HWDGE issues one DMA per 625ns and transfers stream serially),
            # then up weights, conv weights, w_down last (first needed ~58us
            # in).  PE warmup: dummy matmuls on zeroed SBUF fill the ~15us
            # startup DMA wait so the clock gate reaches (and keeps) 2.4 GHz
            # before the first real matmul.
            warm_sb = consts.tile([128, NT], F16)
            nc.gpsimd.memset(warm_sb[:], 0.0)
            warm_ps = ps_g.tile([128, NT], F32, tag="ps")
            for _ in range(14):
                nc.tensor.matmul(
                    warm_ps[:], warm_sb[:, 0:128], warm_sb[:],
                    start=True, stop=True)

            load_x(0)
            # per-chunk hi/lo pairs in the c-major consumption order of
            # tile 0's gate phase, so PE work arrives faster than it is
            # consumed (2.5us of matmuls per 1.6us chunk-pair transfer)
            for c in range(DC2):
                nc.sync.dma_start(out=w_sb["gh"][:, c, :, :],
                                  in_=wr["gh"][:, c, :, :])
                nc.sync.dma_start(out=w_sb["gl"][:, c, :, :],
                                  in_=wr["gl"][:, c, :, :])
            for c in range(DC2):
                nc.sync.dma_start(out=w_sb["uh"][:, c, :, :],
                                  in_=wr["uh"][:, c, :, :])
                nc.sync.dma_start(out=w_sb["ul"][:, c, :, :],
                                  in_=wr["ul"][:, c, :, :])
            nc.sync.dma_start(out=cw_sb[:], in_=cw[:])
            nc.sync.dma_start(out=wdh_sb[:], in_=wdhr[:])
            nc.sync.dma_start(out=wdl_sb[:], in_=wdlr[:])

            g_prev = u_prev = None
            h_tiles = {}

            def gu_matmul(ps, w_hi, w_lo, x_t, fc):
                """One f-chunk of gate/up: 3-term hi/lo fp8 DoubleRow."""
                xh_t, xl_t = x_t
                fs = bass.ts(fc, 128)
                for half in range(2):
                    ts = bass.ts(half, NH)
                    n = 0
                    for rhs_t, lhsT in ((xh_t, w_hi), (xh_t, w_lo),
                                        (xl_t, w_hi)):
                        for c in range(DC2):
                            nc.tensor.matmul(
                                ps[:, ts],
                                lhsT[:, c, :, fs],
                                rhs_t[:, c, :, ts],
                                start=(n == 0),
                                stop=(n == 3 * DC2 - 1),
                                perf_mode=DR,
                            )
                            n += 1

            def gu_matmul_cmajor(ps_list, w_hi, w_lo, x_t):
                """Whole gate/up branch with chunk-major order: all FC psum
                groups open at once, weight chunks consumed in DMA-arrival
                order so tile 0's PE work starts ~4.5us in instead of
                waiting ~16us for the full hi+lo weight tensors."""
                xh_t, xl_t = x_t
                # halves stay sequential per psum tile: a start=True marks the
                # whole 2KB bank pending-zero (ZERO_REGION_SIZE), so half1's
                # chain must not begin until half0's chain is complete.
                for half in range(2):
                    ts = bass.ts(half, NH)
                    for c in range(DC2):
                        for term, (rhs_t, lhsT) in enumerate(
                                ((xh_t, w_hi), (xh_t, w_lo), (xl_t, w_hi))):
                            for fc in range(FC):
                                nc.tensor.matmul(
                                    ps_list[fc][:, ts],
                                    lhsT[:, c, :, bass.ts(fc, 128)],
                                    rhs_t[:, c, :, ts],
                                    start=(c == 0 and term == 0),
                                    stop=(c == DC2 - 1 and term == 2),
                                    perf_mode=DR,
                                )

            def gateup_phase(tt):
                nonlocal g_prev, u_prev
                x_t = x_tiles[tt]
                g_cur = gbpool.tile([128, FC, GROW], F16)
                u_cur = ubpool.tile([128, FC, GROW], F16)
                hs = []
                # conv halo: last 3 tokens of the previous tile (zeros at
                # the start of each batch — causal left pad).
                for buf, prev in ((g_cur, g_prev), (u_cur, u_prev)):
                    if tt % TILES_PER_BATCH == 0:
                        nc.vector.memset(buf[:, :, 0:3], 0.0)
                    else:
                        nc.vector.tensor_copy(
                            out=buf[:, :, 0:3], in_=prev[:, :, NT:NT + 3])
                if tt == 0:
                    # chunk-major startup path: each branch holds all 8 PSUM
                    # banks (2 psg + 2 psu + 4 pso slots) simultaneously
                    for hi, lo, buf in (("gh", "gl", g_cur),
                                        ("uh", "ul", u_cur)):
                        ps_list = []
                        for pool, tag, nb in ((ps_g, "ps", 2), (ps_u, "ps", 2),
                                              (ps_o, "pso", 4)):
                            for _ in range(nb):
                                ps_list.append(pool.tile(
                                    [128, NT], F32, name="ps0", tag=tag))
                        gu_matmul_cmajor(ps_list, w_sb[hi], w_sb[lo], x_t)
                        for fc in range(FC):
                            nc.scalar.mul(buf[:, fc, 3:3 + NT],
                                          ps_list[fc][:], 1.0 / (SX * SW))
                else:
                    # gate and up groups alternate per f-chunk so both
                    # branches' evacs for a chunk-pair land early and the DVE
                    # conv chain starts ~2 chunks in (keeps h ahead of down).
                    for fc in range(FC):
                        for hi, lo, buf, psp in (
                            ("gh", "gl", g_cur, ps_g),
                            ("uh", "ul", u_cur, ps_u),
                        ):
                            ps = psp.tile([128, NT], F32)
                            gu_matmul(ps, w_sb[hi], w_sb[lo], x_t, fc)
                            # evacuate and divide out the fp8 scale product
                            nc.scalar.mul(buf[:, fc, 3:3 + NT], ps[:],
                                          1.0 / (SX * SW))
                for c2 in range(FC2):
                    hh_t = hpool.tile([128, 2, NT], F8)
                    hl_t = hpool.tile([128, 2, NT], F8)
                    for i in range(2):
                        fc = 2 * c2 + i
                        # causal depthwise conv + residual (folded into tap
                        # 3), then h = up_conv * silu(gate_conv).
                        conv_out = []
                        for br, buf in ((0, g_cur), (1, u_cur)):
                            t1 = tpool.tile([128, NT], F16)
                            # All taps on VectorE so ScalarE's stream stays
                            # homogeneous (Copy evacs, then Sigmoids).
                            nc.vector.tensor_scalar(
                                t1[:], buf[:, fc, 1:1 + NT],
                                cw_sb[:, fc, br, 1:2], None, ALU.mult)
                            for k in (0, 2, 3):
                                nc.vector.scalar_tensor_tensor(
                                    out=t1[:], in0=buf[:, fc, k:k + NT],
                                    scalar=cw_sb[:, fc, br, k:k + 1], in1=t1[:],
                                    op0=ALU.mult, op1=ALU.add)
                            conv_out.append(t1)
                        gc, uc = conv_out
                        sg = tpool.tile([128, NT], F16)
                        nc.scalar.activation(
                            out=sg[:], in_=gc[:], func=AF.Sigmoid,
                            bias=zero_b[:, 0:1])
                        nc.vector.tensor_mul(sg[:], sg[:], gc[:])
                        h_t = tpool.tile([128, NT], F16)
                        nc.vector.tensor_mul(h_t[:], uc[:], sg[:])
                        # hi/lo fp8 split of h at scale SH (same scale for
                        # both so all 3 down-proj terms share one PSUM group)
                        nc.scalar.mul(hh_t[:, i, :], h_t[:], SH)
                        nc.vector.scalar_tensor_tensor(
                            out=hl_t[:, i, :], in0=h_t[:], scalar=SH,
                            in1=hh_t[:, i, :],
                            op0=ALU.mult, op1=ALU.subtract)
                    hs.append((hh_t, hl_t))
                g_prev, u_prev = g_cur, u_cur
                h_tiles[tt] = hs

            def down_phase(tt):
                hs = h_tiles.pop(tt)
                # final tile: evacuate + DMA each token-half as soon as its
                # psum chain stops, so the end-of-kernel drain is one
                # half-chain + half-evac + half-DMA instead of a full column
                last = tt == N_TILES - 1
                for dc in range(D // 128):
                    ds = bass.ts(dc, 128)
                    pso = ps_o.tile([128, NT], F32)
                    o_sb = opool.tile([128, NT], F16)
                    for half in range(2):
                        ts = bass.ts(half, NH)
                        n = 0
                        for sel_h, wd_t in ((0, wdh_sb), (0, wdl_sb),
                                            (1, wdh_sb)):
                            for c in range(FC2):
                                nc.tensor.matmul(
                                    pso[:, ts],
                                    wd_t[:, c, :, ds],
                                    hs[c][sel_h][:, :, ts],
                                    start=(n == 0),
                                    stop=(n == 3 * FC2 - 1),
                                    perf_mode=DR,
                                )
                                n += 1
                        if last:
                            nc.scalar.mul(o_sb[:, ts], pso[:, ts],
                                          1.0 / (SH * SW))
                            eng = nc.sync if (2 * dc + half) % 2 == 0 \
                                else nc.scalar
                            eng.dma_start(
                                out=outT[ds, bass.ts(2 * tt + half, NH)],
                                in_=o_sb[:, ts])
                    if not last:
                        nc.scalar.mul(o_sb[:], pso[:], 1.0 / (SH * SW))
                        # alternate the two HWDGE queues so the output drain
                        # is not serialized behind one queue
                        eng = nc.sync if dc % 2 == 0 else nc.scalar
                        eng.dma_start(
                            out=outT[ds, bass.ts(tt, NT)],
                            in_=o_sb[:])

            # Software pipeline: gate/up(tt) is emitted before down(tt-1) so
            # the PE never waits on the conv/act chain of the current tile.
            # x(tt+1) is issued after gateup(tt) so its transfer doesn't cut
            # ahead of the startup weight stream on the serial DMA pipe.
            for tt in range(N_TILES + 1):
                if tt < N_TILES:
                    gateup_phase(tt)
                if tt + 1 <= N_TILES - 1:
                    load_x(tt + 1)
                if tt >= 1:
                    down_phase(tt - 1)

    nc.compile()
    return nc


_NC_CACHE = None


def _get_nc():
    global _NC_CACHE
    if _NC_CACHE is None:
        _NC_CACHE = build_nc()
    return _NC_CACHE


def _split8(a, scale):
    """hi/lo fp8e4 pair of a*scale (shared scale; lo = quantized residual)."""
    sa = a * scale
    hi = sa.astype(E4)
    lo = (sa - hi.astype(np.float32)).astype(E4)
    return hi, lo


def _prep_inputs(x, w_gate, w_up, w_down, conv_w):
    xT = np.ascontiguousarray(x.reshape(TT, D).T)      # [D, TT] fp32
    xh_a, xl_a = _split8(xT, SX)
    # conv weights: [2F, 4] -> per-core [128, FC, 2, 4], residual folded in
    cwf = conv_w.reshape(2, NCORES, FC, 128, 4).astype(np.float32)
    in_maps = []
    for c in range(NCORES):
        fs = slice(c * FC_PER_CORE, (c + 1) * FC_PER_CORE)
        wgh_a, wgl_a = _split8(np.ascontiguousarray(w_gate[fs].T), SW)
        wuh_a, wul_a = _split8(np.ascontiguousarray(w_up[fs].T), SW)
        wdh_a, wdl_a = _split8(np.ascontiguousarray(w_down[:, fs].T), SW)
        cwc = np.ascontiguousarray(
            cwf[:, c].transpose(2, 1, 0, 3))           # [128, FC, 2, 4]
        cwc[:, :, :, 3] += 1.0
        in_maps.append({"xh": xh_a, "xl": xl_a,
                        "wgh": wgh_a, "wgl": wgl_a,
                        "wuh": wuh_a, "wul": wul_a,
                        "wdh": wdh_a, "wdl": wdl_a,
                        "cw": cwc})
    return in_maps


def run_spmd(in_maps, **kwargs):
    nc = _get_nc()
    return run_bass_kernel_spmd(
        nc, in_maps, core_ids=list(range(NCORES)), **kwargs)


def kernel(x, w_gate, w_up, w_down, conv_w):
    in_maps = _prep_inputs(
        np.asarray(x, dtype=np.float32), np.asarray(w_gate, dtype=np.float32),
        np.asarray(w_up, dtype=np.float32),
        np.asarray(w_down, dtype=np.float32),
        np.asarray(conv_w, dtype=np.float32))
    res = run_spmd(in_maps)
    acc = np.zeros((D, TT), np.float32)
    for r in res.results:
        acc += r["outT"].astype(np.float32)
    return np.ascontiguousarray(acc.T).reshape(B, T, D)


# revision 28
# speedup vs baseline: 1.3424x; 1.0104x over previous
"""CanonGLU feedforward layer on 8 TRN2 NeuronCores.

Math (per reference):
    gate = x @ w_gate.T ; up = x @ w_up.T            # [B,T,F]
    gate += causal_dconv(gate, conv_w[:F]) ; up += causal_dconv(up, conv_w[F:])
    out  = (up * silu(gate)) @ w_down.T              # [B,T,D]

Sharding: tensor-parallel over d_ff. Core c owns f-slice [c*1024,(c+1)*1024) of
w_gate/w_up/conv_w (column parallel) and w_down (row parallel); x replicated.
Each core computes a full-shape partial output; the host sums the 8 partials
(the "all-reduce").

Matmuls run as fp8e4 (e4m3) DoubleRow pairs: each instruction contracts two
128-row planes at 0.5 cyc/output-column -- 4x the fp16 row rate.  Accuracy is
recovered with a hi/lo split: every operand X is stored as X_h = fp8(X*s) plus
X_l = fp8(X*s - X_h) at the SAME scale, and each product uses three terms
 A_h@B_h + A_l@B_h + A_h@B_l  (the lo*lo term is ~0.06% and dropped), so each
projection costs 0.75x its fp16 cycle count while the end-to-end error stays
~2e-3 (measured on host emulation).  x and the weights are split on the host;
h = up_c * silu(gate_c) is split on-device (1 scaled copy + 1 subtract per
f-chunk).  PSUM accumulates fp32; conv/silu stay fp16 at true scale (the
fp8 scale product is divided out in the PSUM-evacuation copy).
"""

import numpy as np
import ml_dtypes

import concourse.bass as bass
import concourse.mybir as mybir
import concourse.tile as tile
from concourse import bacc
from concourse.bass_utils import run_bass_kernel_spmd

F8 = mybir.dt.float8e4
F16 = mybir.dt.float16
F32 = mybir.dt.float32
AF = mybir.ActivationFunctionType
ALU = mybir.AluOpType
DR = mybir.MatmulPerfMode.DoubleRow

B, T, D, F = 2, 2048, 2048, 8192
NCORES = 8
FC_PER_CORE = F // NCORES          # 1024 f per core
TT = B * T                         # 4096 tokens total
NT = 512                           # token tile (one PSUM bank of fp32)
NH = NT // 2                       # DoubleRow moving free limit: 2*NH <= 512
N_TILES = TT // NT                 # 8
TILES_PER_BATCH = T // NT          # 4 (conv halo resets at 0 and 4)
DC2 = D // 256                     # 8 d plane-pairs
FC = FC_PER_CORE // 128            # 8 f-chunks per core
FC2 = FC // 2                      # 4 f plane-pairs per core
GROW = NT + 4                      # conv buffer row: 3 halo + 512 data + 1 pad

SX = 16.0                          # x fp8 scale
SW = 512.0                         # weight fp8 scale
SH = 4.0                           # h fp8 scale
E4 = ml_dtypes.float8_e4m3


def build_nc():
    nc = bacc.Bacc(None, target_bir_lowering=False, debug=False)

    # hi/lo fp8 pairs, prepared on the host at a shared power-of-2 scale
    xh = nc.dram_tensor("xh", [D, TT], F8, kind="ExternalInput")
    xl = nc.dram_tensor("xl", [D, TT], F8, kind="ExternalInput")
    wgh = nc.dram_tensor("wgh", [D, FC_PER_CORE], F8, kind="ExternalInput")
    wgl = nc.dram_tensor("wgl", [D, FC_PER_CORE], F8, kind="ExternalInput")
    wuh = nc.dram_tensor("wuh", [D, FC_PER_CORE], F8, kind="ExternalInput")
    wul = nc.dram_tensor("wul", [D, FC_PER_CORE], F8, kind="ExternalInput")
    wdh = nc.dram_tensor("wdh", [FC_PER_CORE, D], F8, kind="ExternalInput")
    wdl = nc.dram_tensor("wdl", [FC_PER_CORE, D], F8, kind="ExternalInput")
    cw = nc.dram_tensor("cw", [128, FC, 2, 4], F32, kind="ExternalInput")
    outT = nc.dram_tensor("outT", [D, TT], F16, kind="ExternalOutput")

    # DoubleRow plane pairing: contraction index = 256*c + 128*i + partition
    xhr = xh.rearrange("(c i p) t -> p c i t", p=128, i=2)
    xlr = xl.rearrange("(c i p) t -> p c i t", p=128, i=2)
    wr = {
        "gh": wgh.rearrange("(c i p) f -> p c i f", p=128, i=2),
        "gl": wgl.rearrange("(c i p) f -> p c i f", p=128, i=2),
        "uh": wuh.rearrange("(c i p) f -> p c i f", p=128, i=2),
        "ul": wul.rearrange("(c i p) f -> p c i f", p=128, i=2),
    }
    wdhr = wdh.rearrange("(c i p) d -> p c i d", p=128, i=2)
    wdlr = wdl.rearrange("(c i p) d -> p c i d", p=128, i=2)

    with tile.TileContext(nc) as tc:
        with (
            tc.tile_pool(name="consts", bufs=1) as consts,
            tc.tile_pool(name="xp", bufs=2) as xpool,
            tc.tile_pool(name="gb", bufs=2) as gbpool,
            tc.tile_pool(name="ub", bufs=2) as ubpool,
            tc.tile_pool(name="hp", bufs=2 * FC2) as hpool,
            tc.tile_pool(name="tp", bufs=4) as tpool,
            tc.tile_pool(name="op", bufs=3) as opool,
            tc.tile_pool(name="psg", bufs=2, space="PSUM") as ps_g,
            tc.tile_pool(name="psu", bufs=2, space="PSUM") as ps_u,
            tc.tile_pool(name="pso", bufs=4, space="PSUM") as ps_o,
        ):
            wgh_sb = consts.tile([128, DC2, 2, FC_PER_CORE], F8)
            wgl_sb = consts.tile([128, DC2, 2, FC_PER_CORE], F8)
            wuh_sb = consts.tile([128, DC2, 2, FC_PER_CORE], F8)
            wul_sb = consts.tile([128, DC2, 2, FC_PER_CORE], F8)
            w_sb = {"gh": wgh_sb, "gl": wgl_sb, "uh": wuh_sb, "ul": wul_sb}
            wdh_sb = consts.tile([128, FC2, 2, D], F8)
            wdl_sb = consts.tile([128, FC2, 2, D], F8)
            cw_sb = consts.tile([128, FC, 2, 4], F32)
            zero_b = consts.tile([128, 1], F32)
            nc.vector.memset(zero_b[:], 0.0)

            x_tiles = {}

            def load_x(tt, chunked=False):
                xh_t = xpool.tile([128, DC2, 2, NT], F8)
                xl_t = xpool.tile([128, DC2, 2, NT], F8)
                if chunked:
                    # tile 0: interleave x chunks with the gate weight
                    # chunk-pairs in c-major consumption order, so the first
                    # matmuls release ~3.7us in instead of ~9.4us
                    for c in range(DC2):
                        nc.sync.dma_start(out=xh_t[:, c, :, :],
                                          in_=xhr[:, c, :, bass.ts(tt, NT)])
                        nc.sync.dma_start(out=xl_t[:, c, :, :],
                                          in_=xlr[:, c, :, bass.ts(tt, NT)])
                        nc.sync.dma_start(out=w_sb["gh"][:, c, :, :],
                                          in_=wr["gh"][:, c, :, :])
                        nc.sync.dma_start(out=w_sb["gl"][:, c, :, :],
                                          in_=wr["gl"][:, c, :, :])
                else:
                    nc.sync.dma_start(out=xh_t[:],
                                      in_=xhr[:, :, :, bass.ts(tt, NT)])
                    nc.sync.dma_start(out=xl_t[:],
                                      in_=xlr[:, :, :, bass.ts(tt, NT)])
                x_tiles[tt] = (xh_t, xl_t)

            # DMA order at startup: x(0) and the gate hi/lo weights first
            # (first gate group is gated by these — big transfers, since the
            # HWDGE issues one DMA per 625ns and transfers stream serially),
            # then up weights, conv weights, w_down last (first needed ~58us
            # in).  PE warmup: dummy matmuls on zeroed SBUF fill the ~15us
            # startup DMA wait so the clock gate reaches (and keeps) 2.4 GHz
            # before the first real matmul.
            warm_sb = consts.tile([128, NH], F16)
            nc.vector.memset(warm_sb[:], 0.0)
            warm_ps = ps_g.tile([128, NT], F32, tag="ps")
            for _ in range(16):
                nc.tensor.matmul(
                    warm_ps[:, 0:NH], warm_sb[:, 0:128], warm_sb[:],
                    start=True, stop=True)

            # x and gate-weight chunks interleaved in c-major consumption
            # order (PE work arrives faster than it is consumed: 2.56us of
            # matmuls per 2.1us chunk group), then up weights per-chunk
            load_x(0, chunked=True)
            for c in range(DC2):
                nc.sync.dma_start(out=w_sb["uh"][:, c, :, :],
                                  in_=wr["uh"][:, c, :, :])
                nc.sync.dma_start(out=w_sb["ul"][:, c, :, :],
                                  in_=wr["ul"][:, c, :, :])
            # x(1) ahead of w_down in the serial transfer stream: gateup(1)
            # needs it ~45us in, w_down isn't read until down(0) ~70us in
            load_x(1)
            nc.sync.dma_start(out=cw_sb[:], in_=cw[:])
            nc.sync.dma_start(out=wdh_sb[:], in_=wdhr[:])
            nc.sync.dma_start(out=wdl_sb[:], in_=wdlr[:])

            g_prev = u_prev = None
            h_tiles = {}

            def gu_matmul(ps, w_hi, w_lo, x_t, fc):
                """One f-chunk of gate/up: 3-term hi/lo fp8 DoubleRow."""
                xh_t, xl_t = x_t
                fs = bass.ts(fc, 128)
                for half in range(2):
                    ts = bass.ts(half, NH)
                    n = 0
                    for rhs_t, lhsT in ((xh_t, w_hi), (xh_t, w_lo),
                                        (xl_t, w_hi)):
                        for c in range(DC2):
                            nc.tensor.matmul(
                                ps[:, ts],
                                lhsT[:, c, :, fs],
                                rhs_t[:, c, :, ts],
                                start=(n == 0),
                                stop=(n == 3 * DC2 - 1),
                                perf_mode=DR,
                            )
                            n += 1

            def gu_matmul_cmajor(ps_list, w_hi, w_lo, x_t):
                """Whole gate/up branch with chunk-major order: all FC psum
                groups open at once, weight chunks consumed in DMA-arrival
                order so tile 0's PE work starts ~4.5us in instead of
                waiting ~16us for the full hi+lo weight tensors.

                Both token halves interleave inside each bank: start=True is
                emitted only on the bank's first matmul — its whole-2KB
                pending-zero mark (ZERO_REGION_SIZE) covers half1's region in
                the simulator, while on hardware (per-element zeroing) half1
                instead accumulates onto the explicit memset below."""
                xh_t, xl_t = x_t
                for c in range(DC2):
                    for term, (rhs_t, lhsT) in enumerate(
                            ((xh_t, w_hi), (xh_t, w_lo), (xl_t, w_hi))):
                        for fc in range(FC):
                            for half in range(2):
                                ts = bass.ts(half, NH)
                                nc.tensor.matmul(
                                    ps_list[fc][:, ts],
                                    lhsT[:, c, :, bass.ts(fc, 128)],
                                    rhs_t[:, c, :, ts],
                                    start=(c == 0 and term == 0
                                           and half == 0),
                                    stop=(c == DC2 - 1 and term == 2),
                                    perf_mode=DR,
                                )

            def gateup_phase(tt):
                nonlocal g_prev, u_prev
                x_t = x_tiles[tt]
                g_cur = gbpool.tile([128, FC, GROW], F16)
                u_cur = ubpool.tile([128, FC, GROW], F16)
                hs = []
                # conv halo: last 3 tokens of the previous tile (zeros at
                # the start of each batch — causal left pad).
                for buf, prev in ((g_cur, g_prev), (u_cur, u_prev)):
                    if tt % TILES_PER_BATCH == 0:
                        nc.vector.memset(buf[:, :, 0:3], 0.0)
                    else:
                        nc.vector.tensor_copy(
                            out=buf[:, :, 0:3], in_=prev[:, :, NT:NT + 3])
                if tt == 0:
                    # chunk-major startup path: each branch holds all 8 PSUM
                    # banks (2 psg + 2 psu + 4 pso slots) simultaneously.
                    # Half1 psum regions are zeroed on ScalarE (their first
                    # matmul carries start=False: the bank-wide pending-zero
                    # from half0's start covers them in the simulator; on HW
                    # they accumulate onto these zeros).  The up-branch
                    # memzeros interleave with the gate evacs so up's first
                    # half1 matmuls aren't queued behind all 8 evacs.
                    def alloc8():
                        ps_list = []
                        for pool, tag, nb in ((ps_g, "ps", 2), (ps_u, "ps", 2),
                                              (ps_o, "pso", 4)):
                            for _ in range(nb):
                                ps_list.append(pool.tile(
                                    [128, NT], F32, name="ps0", tag=tag))
                        return ps_list

                    ps_gate = alloc8()
                    for fc in range(FC):
                        nc.scalar.memzero(ps_gate[fc][:, NH:NT])
                    gu_matmul_cmajor(ps_gate, w_sb["gh"], w_sb["gl"], x_t)
                    ps_up = alloc8()
                    for fc in range(FC):
                        nc.scalar.mul(g_cur[:, fc, 3:3 + NT],
                                      ps_gate[fc][:], 1.0 / (SX * SW))
                        nc.scalar.memzero(ps_up[fc][:, NH:NT])
                    gu_matmul_cmajor(ps_up, w_sb["uh"], w_sb["ul"], x_t)
                    for fc in range(FC):
                        nc.scalar.mul(u_cur[:, fc, 3:3 + NT],
                                      ps_up[fc][:], 1.0 / (SX * SW))
                else:
                    # gate and up groups alternate per f-chunk so both
                    # branches' evacs for a chunk-pair land early and the DVE
                    # conv chain starts ~2 chunks in (keeps h ahead of down).
                    for fc in range(FC):
                        for hi, lo, buf, psp in (
                            ("gh", "gl", g_cur, ps_g),
                            ("uh", "ul", u_cur, ps_u),
                        ):
                            ps = psp.tile([128, NT], F32)
                            gu_matmul(ps, w_sb[hi], w_sb[lo], x_t, fc)
                            # evacuate and divide out the fp8 scale product
                            nc.scalar.mul(buf[:, fc, 3:3 + NT], ps[:],
                                          1.0 / (SX * SW))
                for c2 in range(FC2):
                    hh_t = hpool.tile([128, 2, NT], F8)
                    hl_t = hpool.tile([128, 2, NT], F8)
                    for i in range(2):
                        fc = 2 * c2 + i
                        # causal depthwise conv + residual (folded into tap
                        # 3), then h = up_conv * silu(gate_conv).
                        conv_out = []
                        for br, buf in ((0, g_cur), (1, u_cur)):
                            t1 = tpool.tile([128, NT], F16)
                            # All taps on VectorE so ScalarE's stream stays
                            # homogeneous (Copy evacs, then Sigmoids).
                            nc.vector.tensor_scalar(
                                t1[:], buf[:, fc, 1:1 + NT],
                                cw_sb[:, fc, br, 1:2], None, ALU.mult)
                            for k in (0, 2, 3):
                                nc.vector.scalar_tensor_tensor(
                                    out=t1[:], in0=buf[:, fc, k:k + NT],
                                    scalar=cw_sb[:, fc, br, k:k + 1], in1=t1[:],
                                    op0=ALU.mult, op1=ALU.add)
                            conv_out.append(t1)
                        gc, uc = conv_out
                        sg = tpool.tile([128, NT], F16)
                        nc.scalar.activation(
                            out=sg[:], in_=gc[:], func=AF.Sigmoid,
                            bias=zero_b[:, 0:1])
                        nc.vector.tensor_mul(sg[:], sg[:], gc[:])
                        h_t = tpool.tile([128, NT], F16)
                        nc.vector.tensor_mul(h_t[:], uc[:], sg[:])
                        # hi/lo fp8 split of h at scale SH (same scale for
                        # both so all 3 down-proj terms share one PSUM group)
                        nc.scalar.mul(hh_t[:, i, :], h_t[:], SH)
                        nc.vector.scalar_tensor_tensor(
                            out=hl_t[:, i, :], in0=h_t[:], scalar=SH,
                            in1=hh_t[:, i, :],
                            op0=ALU.mult, op1=ALU.subtract)
                    hs.append((hh_t, hl_t))
                g_prev, u_prev = g_cur, u_cur
                h_tiles[tt] = hs

            def down_phase(tt):
                hs = h_tiles.pop(tt)
                for dc in range(D // 128):
                    ds = bass.ts(dc, 128)
                    pso = ps_o.tile([128, NT], F32)
                    for half in range(2):
                        ts = bass.ts(half, NH)
                        n = 0
                        for sel_h, wd_t in ((0, wdh_sb), (0, wdl_sb),
                                            (1, wdh_sb)):
                            for c in range(FC2):
                                nc.tensor.matmul(
                                    pso[:, ts],
                                    wd_t[:, c, :, ds],
                                    hs[c][sel_h][:, :, ts],
                                    start=(n == 0),
                                    stop=(n == 3 * FC2 - 1),
                                    perf_mode=DR,
                                )
                                n += 1
                    o_sb = opool.tile([128, NT], F16)
                    nc.scalar.mul(o_sb[:], pso[:], 1.0 / (SH * SW))
                    # alternate the two HWDGE queues so the final tile's
                    # output drain is not serialized behind one queue
                    eng = nc.sync if dc % 2 == 0 else nc.scalar
                    eng.dma_start(
                        out=outT[ds, bass.ts(tt, NT)],
                        in_=o_sb[:])

            # Software pipeline: gate/up(tt) is emitted before down(tt-1) so
            # the PE never waits on the conv/act chain of the current tile.
            # x(tt+1) is issued after gateup(tt) so its transfer doesn't cut
            # ahead of the startup weight stream on the serial DMA pipe.
            for tt in range(N_TILES + 1):
                if tt < N_TILES:
                    gateup_phase(tt)
                if 1 <= tt + 1 <= N_TILES - 1 and tt >= 1:  # x(1) loads above
                    load_x(tt + 1)
                if tt >= 1:
                    down_phase(tt - 1)

    nc.compile()
    return nc


_NC_CACHE = None


def _get_nc():
    global _NC_CACHE
    if _NC_CACHE is None:
        _NC_CACHE = build_nc()
    return _NC_CACHE


def _split8(a, scale):
    """hi/lo fp8e4 pair of a*scale (shared scale; lo = quantized residual)."""
    sa = a * scale
    hi = sa.astype(E4)
    lo = (sa - hi.astype(np.float32)).astype(E4)
    return hi, lo


def _prep_inputs(x, w_gate, w_up, w_down, conv_w):
    xT = np.ascontiguousarray(x.reshape(TT, D).T)      # [D, TT] fp32
    xh_a, xl_a = _split8(xT, SX)
    # conv weights: [2F, 4] -> per-core [128, FC, 2, 4], residual folded in
    cwf = conv_w.reshape(2, NCORES, FC, 128, 4).astype(np.float32)
    in_maps = []
    for c in range(NCORES):
        fs = slice(c * FC_PER_CORE, (c + 1) * FC_PER_CORE)
        wgh_a, wgl_a = _split8(np.ascontiguousarray(w_gate[fs].T), SW)
        wuh_a, wul_a = _split8(np.ascontiguousarray(w_up[fs].T), SW)
        wdh_a, wdl_a = _split8(np.ascontiguousarray(w_down[:, fs].T), SW)
        cwc = np.ascontiguousarray(
            cwf[:, c].transpose(2, 1, 0, 3))           # [128, FC, 2, 4]
        cwc[:, :, :, 3] += 1.0
        in_maps.append({"xh": xh_a, "xl": xl_a,
                        "wgh": wgh_a, "wgl": wgl_a,
                        "wuh": wuh_a, "wul": wul_a,
                        "wdh": wdh_a, "wdl": wdl_a,
                        "cw": cwc})
    return in_maps


def run_spmd(in_maps, **kwargs):
    nc = _get_nc()
    return run_bass_kernel_spmd(
        nc, in_maps, core_ids=list(range(NCORES)), **kwargs)


def kernel(x, w_gate, w_up, w_down, conv_w):
    in_maps = _prep_inputs(
        np.asarray(x, dtype=np.float32), np.asarray(w_gate, dtype=np.float32),
        np.asarray(w_up, dtype=np.float32),
        np.asarray(w_down, dtype=np.float32),
        np.asarray(conv_w, dtype=np.float32))
    res = run_spmd(in_maps)
    acc = np.zeros((D, TT), np.float32)
    for r in res.results:
        acc += r["outT"].astype(np.float32)
    return np.ascontiguousarray(acc.T).reshape(B, T, D)
